# revision 5
# baseline (speedup 1.0000x reference)
"""BatchGAT (2-layer, 8-head GAT over 32 graphs of 512 nodes) on 8 TRN2 NeuronCores.

Strategy: data-parallel over the batch — each of the 8 cores processes 4 graphs.
Per graph/layer, the masked-softmax attention is computed in transposed layout
E^T[j, i] = adj[i,j] * exp(leaky_relu(s_i + d_j)) using the identity

    exp(leaky_relu(x, a)) = max(exp(x), exp(a*x))        (0 < a < 1)
    E^T[j,i] = max(es_i * r_j, es2_i) * ed2_j * adj[j,i]

with es = exp(s), es2 = exp(0.2 s) (row profiles broadcast across partitions via
DMA), r = exp(0.8 d), ed2 = exp(0.2 d) (per-partition scalars). That makes the
whole E build two fused scalar_tensor_tensor passes per (head, node-tile), split
across the Vector and GpSimd engines. Aggregation out = A @ hp runs on the
TensorEngine with E^T tiles as lhsT and hp (augmented with a ones column) as
rhs, so the softmax denominator falls out as output column 64 and the
normalization is a per-partition scalar multiply fused into the PSUM->SBUF
evacuation on the Scalar engine.
"""

import sys

if "/opt/trn_rl_repo" not in sys.path:
    sys.path.insert(0, "/opt/trn_rl_repo")

import numpy as np
import ml_dtypes

import concourse.bacc as bacc
import concourse.mybir as mybir
from concourse import tile
from concourse.bass_utils import run_bass_kernel_spmd
from concourse.alu_op_type import AluOpType

F32 = mybir.dt.float32
BF16 = mybir.dt.bfloat16
BF = ml_dtypes.bfloat16

B, N, FIN, H, F = 32, 512, 64, 8, 64
NCORES = 8
G = B // NCORES          # graphs per core
NT = N // 128            # node tiles
C1 = H * F               # layer-1 input features (512)

_cached = {}


def _build():
    nc = bacc.Bacc("TRN2", target_bir_lowering=False, debug=False)

    xT = nc.dram_tensor("xT", [G, FIN, N], F32, kind="ExternalInput").ap()
    adjb = nc.dram_tensor("adjb", [G, N, N], BF16, kind="ExternalInput").ap()
    w0d = nc.dram_tensor("w0d", [FIN, F + H], F32, kind="ExternalInput").ap()
    was0 = nc.dram_tensor("was0", [FIN, H], F32, kind="ExternalInput").ap()
    w1d = nc.dram_tensor("w1d", [C1, F + H], BF16, kind="ExternalInput").ap()
    was1 = nc.dram_tensor("was1", [C1, H], BF16, kind="ExternalInput").ap()
    out = nc.dram_tensor("out", [G, N, F], F32, kind="ExternalOutput").ap()

    with tile.TileContext(nc) as tc:
        _emit(nc, tc, xT, adjb, w0d, was0, w1d, was1, out)
    nc.compile()
    return nc


def _emit(nc, tc, xT, adjb, w0d, was0, w1d, was1, out):
    from contextlib import ExitStack

    ctx = ExitStack()
    with ctx:
        # weight tiles (loaded once, reused for all graphs)
        wpool = ctx.enter_context(tc.tile_pool(name="weights", bufs=1))
        w0d_sb = wpool.tile([FIN, F + H], F32, tag="w0d")
        nc.sync.dma_start(w0d_sb[:], w0d[:])
        was0_sb = wpool.tile([FIN, H], F32, tag="was0")
        nc.sync.dma_start(was0_sb[:], was0[:])
        # w1d rows c=0..511 -> [p, ct, f] with c = ct*128 + p
        w1d_sb = wpool.tile([128, NT, F + H], BF16, tag="w1d")
        nc.sync.dma_start(w1d_sb[:], w1d.rearrange("(c p) f -> p c f", p=128))
        was1_sb = wpool.tile([128, NT, H], BF16, tag="was1")
        nc.sync.dma_start(was1_sb[:], was1.rearrange("(c p) f -> p c f", p=128))

        # pools
        xt_pool = ctx.enter_context(tc.tile_pool(name="xt", bufs=2))
        adj_pool = ctx.enter_context(tc.tile_pool(name="adj", bufs=2 * NT))
        hp_pool = ctx.enter_context(tc.tile_pool(name="hp", bufs=2 * NT))
        dsc_pool = ctx.enter_context(tc.tile_pool(name="dscal", bufs=2 * NT))
        esr_pool = ctx.enter_context(tc.tile_pool(name="esr", bufs=4))
        esbc_pool = ctx.enter_context(tc.tile_pool(name="esbc", bufs=10))
        et_pool = ctx.enter_context(tc.tile_pool(name="et", bufs=2 * NT))
        u_pool = ctx.enter_context(tc.tile_pool(name="u", bufs=4))
        den_pool = ctx.enter_context(tc.tile_pool(name="den", bufs=4))
        x1_pool = ctx.enter_context(tc.tile_pool(name="x1", bufs=2 * NT))
        x1t_pool = ctx.enter_context(tc.tile_pool(name="x1t", bufs=2 * NT))
        post_pool = ctx.enter_context(tc.tile_pool(name="post", bufs=4))
        out_pool = ctx.enter_context(tc.tile_pool(name="out", bufs=4))

        esd_pool = ctx.enter_context(tc.tile_pool(name="esd", bufs=2, space="DRAM"))
        ps_proj = ctx.enter_context(tc.tile_pool(name="ps_proj", bufs=2, space="PSUM"))
        ps_s = ctx.enter_context(tc.tile_pool(name="ps_s", bufs=2, space="PSUM"))
        ps_agg = ctx.enter_context(tc.tile_pool(name="ps_agg", bufs=4, space="PSUM"))

        for g in range(G):
            # ---- loads ----
            xt = xt_pool.tile([FIN, N], F32, tag="xt")
            nc.sync.dma_start(xt[:], xT[g])
            adj_t = []
            for jt in range(NT):
                a = adj_pool.tile([128, N], BF16, tag="adj")
                nc.sync.dma_start(a[:], adjb[g, jt * 128:(jt + 1) * 128, :])
                adj_t.append(a)

            x1t = None
            for layer in range(2):
                # ---- projections: hp_aug, r, ed2, s-rows ----
                hp_aug, r_sc, ed2_sc = [], [], []
                for jt in range(NT):
                    pp = ps_proj.tile([128, F + H], F32, tag="proj")
                    if layer == 0:
                        nc.tensor.matmul(
                            pp[:], xt[:, jt * 128:(jt + 1) * 128], w0d_sb[:],
                            start=True, stop=True)
                    else:
                        for ct in range(NT):
                            nc.tensor.matmul(
                                pp[:], x1t[ct][:, jt * 128:(jt + 1) * 128],
                                w1d_sb[:, ct, :],
                                start=(ct == 0), stop=(ct == NT - 1))
                    ha = hp_pool.tile([128, F + 1], BF16, tag="hp")
                    nc.scalar.copy(ha[:, 0:F], pp[:, 0:F])
                    # ones column; 8.0 on layer 1 folds the head-mean into 1/den
                    nc.vector.memset(ha[:, F:F + 1], 1.0 if layer == 0 else 8.0)
                    hp_aug.append(ha)
                    rr = dsc_pool.tile([128, H], F32, tag="rsc")
                    nc.scalar.activation(rr[:], pp[:, F:F + H],
                                         mybir.ActivationFunctionType.Exp, scale=0.8)
                    r_sc.append(rr)
                    ee = dsc_pool.tile([128, H], F32, tag="ed2")
                    nc.scalar.activation(ee[:], pp[:, F:F + H],
                                         mybir.ActivationFunctionType.Exp, scale=0.2)
                    ed2_sc.append(ee)

                ps = ps_s.tile([H, N], F32, tag="s")
                if layer == 0:
                    nc.tensor.matmul(ps[:], was0_sb[:], xt[:], start=True, stop=True)
                else:
                    for ct in range(NT):
                        nc.tensor.matmul(ps[:], was1_sb[:, ct, :], x1t[ct][:],
                                         start=(ct == 0), stop=(ct == NT - 1))
                esr = esr_pool.tile([H, 2 * N], BF16, tag="esr")
                nc.scalar.activation(esr[:, 0:N], ps[:],
                                     mybir.ActivationFunctionType.Exp, scale=1.0)
                nc.scalar.activation(esr[:, N:2 * N], ps[:],
                                     mybir.ActivationFunctionType.Exp, scale=0.2)
                esd = esd_pool.tile([H, 2 * N], BF16, tag="esd")
                nc.sync.dma_start(esd[:], esr[:])

                es_bc, es2_bc = [], []
                for h in range(H):
                    eb = esbc_pool.tile([128, N], BF16, tag="esbc")
                    nc.sync.dma_start(
                        eb[:], esd[h:h + 1, 0:N].partition_broadcast(128))
                    es_bc.append(eb)
                    eb2 = esbc_pool.tile([128, N], BF16, tag="es2bc")
                    nc.sync.dma_start(
                        eb2[:], esd[h:h + 1, N:2 * N].partition_broadcast(128))
                    es2_bc.append(eb2)

                # destination of normalized per-head outputs
                stacked = []
                for it in range(NT):
                    st = x1_pool.tile([128, C1], BF16, tag="x1pre")
                    stacked.append(st)

                # ---- per head: E build + aggregation + normalize ----
                for h in range(H):
                    et_h, hps_h = [], []
                    for jt in range(NT):
                        # fold ed2 into hp: hp'[j,:] = hp_aug[j,:] * exp(0.2 d_j)
                        hps = hp_pool.tile([128, F + 1], BF16, tag="hps")
                        nc.vector.tensor_scalar_mul(
                            hps[:], hp_aug[jt][:], ed2_sc[jt][:, h:h + 1])
                        hps_h.append(hps)
                        # E^T = (es_i * r_j  max  es2_i) * adj
                        u = u_pool.tile([128, N], BF16, tag="u")
                        nc.vector.scalar_tensor_tensor(
                            u[:], es_bc[h][:], r_sc[jt][:, h:h + 1], es2_bc[h][:],
                            AluOpType.mult, AluOpType.max)
                        eng = nc.gpsimd if (h * NT + jt) % 4 != 0 else nc.vector
                        et = et_pool.tile([128, N], BF16, tag="et")
                        eng.tensor_tensor(et[:], u[:], adj_t[jt][:], AluOpType.mult)
                        et_h.append(et)

                    po = ps_agg.tile([128, NT * (F + 1)], F32, tag="agg")
                    for it in range(NT):
                        for jt in range(NT):
                            nc.tensor.matmul(
                                po[:, it * (F + 1):(it + 1) * (F + 1)],
                                et_h[jt][:, it * 128:(it + 1) * 128],
                                hps_h[jt][:],
                                start=(jt == 0), stop=(jt == NT - 1))

                    den = den_pool.tile([128, NT], F32, tag="den")
                    # gather the 4 denominator columns (col 64 of each it-chunk)
                    nc.scalar.copy(
                        den[:], po[:, F:F + 1 + (NT - 1) * (F + 1):F + 1])
                    rd = den_pool.tile([128, NT], F32, tag="rd")
                    nc.vector.reciprocal(rd[:], den[:])
                    for it in range(NT):
                        # evacuate + normalize: out = psum * (1/den), cast bf16
                        nc.scalar.activation(
                            stacked[it][:, h * F:(h + 1) * F],
                            po[:, it * (F + 1):it * (F + 1) + F],
                            mybir.ActivationFunctionType.Copy,
                            scale=rd[:, it:it + 1])

                # ---- post ----
                if layer == 0:
                    x1t = [x1t_pool.tile([128, N], BF16, tag="x1t",
                                         name=f"x1t_{g}_{ct}")
                           for ct in range(NT)]
                    for it in range(NT):
                        t = post_pool.tile([128, C1], BF16, tag="expt")
                        nc.scalar.activation(t[:], stacked[it][:],
                                             mybir.ActivationFunctionType.Exp)
                        u2 = post_pool.tile([128, C1], BF16, tag="u2")
                        nc.vector.tensor_scalar_add(u2[:], t[:], -1.0)
                        x1e = post_pool.tile([128, C1], BF16, tag="x1e")
                        # elu(x) = min(relu(x), exp(x) - 1)
                        nc.vector.scalar_tensor_tensor(
                            x1e[:], stacked[it][:], 0.0, u2[:],
                            AluOpType.max, AluOpType.min)
                        for ct in range(NT):
                            nc.sync.dma_start_transpose(
                                x1t[ct][:, it * 128:(it + 1) * 128],
                                x1e[:, ct * 128:(ct + 1) * 128])
                else:
                    for it in range(NT):
                        t1 = out_pool.tile([128, C1 // 2], F32, tag="t1")
                        nc.vector.tensor_add(t1[:], stacked[it][:, 0:256],
                                             stacked[it][:, 256:512])
                        t2 = out_pool.tile([128, C1 // 4], F32, tag="t2")
                        nc.vector.tensor_add(t2[:], t1[:, 0:128], t1[:, 128:256])
                        oo = out_pool.tile([128, F], F32, tag="oo")
                        nc.vector.tensor_add(oo[:], t2[:, 0:64], t2[:, 64:128])
                        nc.sync.dma_start(out[g, it * 128:(it + 1) * 128, :], oo[:])


def _get_nc():
    if "nc" not in _cached:
        _cached["nc"] = _build()
    return _cached["nc"]


def _prep_inputs(x, adj, W0, a_src0, a_dst0, W1, a_src1, a_dst1):
    x = np.asarray(x, np.float32)
    adj = np.array(adj, np.float32, copy=True)
    idx = np.arange(N)
    adj[:, idx, idx] = 1.0  # self loops (reference mutates adj the same way)
    xT = np.ascontiguousarray(x.transpose(0, 2, 1))          # [B, 64, 512]
    adjb = adj.astype(BF)                                     # exact: 0/1
    W0 = np.asarray(W0, np.float32)
    W1 = np.asarray(W1, np.float32)
    w0d = np.concatenate([W0, W0 @ np.asarray(a_dst0, np.float32)], axis=1)
    was0 = W0 @ np.asarray(a_src0, np.float32)
    w1d = np.concatenate([W1, W1 @ np.asarray(a_dst1, np.float32)], axis=1).astype(BF)
    was1 = (W1 @ np.asarray(a_src1, np.float32)).astype(BF)
    in_maps = []
    for c in range(NCORES):
        sl = slice(c * G, (c + 1) * G)
        in_maps.append(dict(
            xT=np.ascontiguousarray(xT[sl]),
            adjb=np.ascontiguousarray(adjb[sl]),
            w0d=w0d, was0=was0, w1d=w1d, was1=was1,
        ))
    return in_maps


def run(inputs, **kw):
    """Build+run; returns (output [B,N,F] float32, BassKernelResults)."""
    nc = _get_nc()
    in_maps = _prep_inputs(
        inputs["x"], inputs["adj"], inputs["W0"], inputs["a_src0"],
        inputs["a_dst0"], inputs["W1"], inputs["a_src1"], inputs["a_dst1"])
    res = run_bass_kernel_spmd(nc, in_maps, list(range(NCORES)), **kw)
    outs = [res.results[c]["out"].reshape(G, N, F) for c in range(NCORES)]
    return np.concatenate(outs, axis=0).astype(np.float32), res


def kernel(**inputs):
    out, _ = run(inputs)
    return out


if __name__ == "__main__":
    rng = np.random.default_rng(0)
    ins = dict(
        x=rng.standard_normal((B, N, FIN), dtype=np.float32),
        adj=(rng.random((B, N, N)) < 0.05).astype(np.float32),
        b_idx=np.repeat(np.arange(B, dtype=np.int32), N),
        r_idx=np.tile(np.arange(N, dtype=np.int32), B),
        W0=rng.standard_normal((FIN, F), dtype=np.float32) / 8,
        a_src0=rng.standard_normal((F, H), dtype=np.float32) / 8,
        a_dst0=rng.standard_normal((F, H), dtype=np.float32) / 8,
        W1=rng.standard_normal((C1, F), dtype=np.float32) / 22.6,
        a_src1=rng.standard_normal((F, H), dtype=np.float32) / 8,
        a_dst1=rng.standard_normal((F, H), dtype=np.float32) / 8,
    )
    ins["adj"] = np.maximum(ins["adj"], ins["adj"].transpose(0, 2, 1))
    o = kernel(**ins)
    print("out", o.shape, o.dtype, float(np.abs(o).max()))


# revision 16
# speedup vs baseline: 1.1725x; 1.1725x over previous
"""BatchGAT (2-layer, 8-head GAT over 32 graphs of 512 nodes) on 8 TRN2 NeuronCores.

Data-parallel over the batch: each core processes 4 graphs. Per graph/layer the
masked softmax attention is built in transposed layout E^T[j, i] (j = neighbor on
partitions) so the aggregation A @ hp runs on TensorE with E^T as lhsT and hp
(plus a ones column) as rhs — the softmax denominator falls out as an output
column and normalization is a per-partition scale fused into PSUM evacuation.

E^T[j,i] = exp(leaky_relu(s_i + d_j))*adj is built by two engine paths, load
balanced across Vector / GpSimd / Scalar / Tensor:

 D-path (DVE): exp(leaky(x)) = max(es_i*r_j, es2_i) * ed2_j with es=exp(s),
   es2=exp(0.2 s) broadcast row tiles, r=exp(0.8 d), ed2=exp(0.2 d) per-partition
   scalars; the ed2 factor is folded into a cheap per-head rescale of the [128,65]
   hp tile (it rides through the aggregation), and the adjacency mask is applied
   as min(u, adjP) with adjP in {BIG, 0}.

 A-path (ACT): logits x = s_i + d_j via a K=2 matmul, plus an identity-weight
   matmul accumulating an additive mask adjM in {0, -BIG} into the same PSUM;
   then E = max(exp(x), exp(0.2 x)) — two ACT exps from PSUM (one table set) and
   one DVE max.
"""

import sys

if "/opt/trn_rl_repo" not in sys.path:
    sys.path.insert(0, "/opt/trn_rl_repo")

import numpy as np
import ml_dtypes

import concourse.bacc as bacc
import concourse.mybir as mybir
from concourse import tile
from concourse.bass_utils import run_bass_kernel_spmd
from concourse.alu_op_type import AluOpType

F32 = mybir.dt.float32
BF16 = mybir.dt.bfloat16
BF = ml_dtypes.bfloat16
AF = mybir.ActivationFunctionType

B, N, FIN, H, F = 32, 512, 64, 8, 64
NCORES = 8
G = B // NCORES          # graphs per core
NT = N // 128            # node tiles
C1 = H * F               # layer-1 input features (512)
BIG = 30000.0

_cached = {}


def _a_path(h, jt):
    """Which (head, j-tile) pairs use the ACT/PE logit path."""
    return ((h * NT + jt) * 5) % 16 < 5


def _gps_mask(h, jt):
    """Which D-pairs run the mask min on GpSimd."""
    return ((h * NT + jt) * 7) % 16 < 9


def _build():
    nc = bacc.Bacc("TRN2", target_bir_lowering=False, debug=False)

    xT = nc.dram_tensor("xT", [G, FIN, N], F32, kind="ExternalInput").ap()
    adjP = nc.dram_tensor("adjP", [G, N, N], BF16, kind="ExternalInput").ap()  # {0,1}
    adjM = nc.dram_tensor("adjM", [G, N, N], BF16, kind="ExternalInput").ap()
    ident = nc.dram_tensor("ident", [128, 128], BF16, kind="ExternalInput").ap()
    w0d = nc.dram_tensor("w0d", [FIN, F + 2 * H], F32, kind="ExternalInput").ap()
    w1d = nc.dram_tensor("w1d", [C1, F + 2 * H], BF16, kind="ExternalInput").ap()
    out = nc.dram_tensor("out", [G, N, F], F32, kind="ExternalOutput").ap()

    with tile.TileContext(nc) as tc:
        _emit(nc, tc, xT, adjP, adjM, ident, w0d, w1d, out)
    nc.compile()
    return nc


def _emit(nc, tc, xT, adjP, adjM, ident, w0d, w1d, out):
    from contextlib import ExitStack

    ctx = ExitStack()
    with ctx:
        # weights: [W | W@a_dst | W@a_src] -> proj matmul yields [hp | d | s]
        wpool = ctx.enter_context(tc.tile_pool(name="weights", bufs=1))
        w0d_sb = wpool.tile([FIN, F + 2 * H], F32, tag="w0d")
        nc.sync.dma_start(w0d_sb[:], w0d[:])
        w1d_sb = wpool.tile([128, NT, F + 2 * H], BF16, tag="w1d")
        nc.sync.dma_start(w1d_sb[:], w1d.rearrange("(c p) f -> p c f", p=128))
        id_sb = wpool.tile([128, 128], BF16, tag="ident")
        nc.sync.dma_start(id_sb[:], ident[:])
        ones_row = wpool.tile([1, N], BF16, tag="ones_row")
        nc.vector.memset(ones_row[:], 1.0)

        xt_pool = ctx.enter_context(tc.tile_pool(name="xt", bufs=2))
        adj_pool = ctx.enter_context(tc.tile_pool(name="adj", bufs=2 * NT))
        hp_pool = ctx.enter_context(tc.tile_pool(name="hp", bufs=2 * NT))
        hps_pool = ctx.enter_context(tc.tile_pool(name="hps", bufs=2 * NT))
        dsc_pool = ctx.enter_context(tc.tile_pool(name="dscal", bufs=2 * NT))
        esr_pool = ctx.enter_context(tc.tile_pool(name="esr", bufs=4))
        esbc_pool = ctx.enter_context(tc.tile_pool(name="esbc", bufs=2 * H))
        et_pool = ctx.enter_context(tc.tile_pool(name="et", bufs=2 * NT))
        u_pool = ctx.enter_context(tc.tile_pool(name="u", bufs=6))
        den_pool = ctx.enter_context(tc.tile_pool(name="den", bufs=4))
        x1_pool = ctx.enter_context(tc.tile_pool(name="x1", bufs=2 * NT))
        x1t_pool = ctx.enter_context(tc.tile_pool(name="x1t", bufs=2 * NT))
        post_pool = ctx.enter_context(tc.tile_pool(name="post", bufs=4))
        out_pool = ctx.enter_context(tc.tile_pool(name="out", bufs=4))
        esd_pool = ctx.enter_context(tc.tile_pool(name="esd", bufs=2, space="DRAM"))

        ps_proj = ctx.enter_context(tc.tile_pool(name="ps_proj", bufs=2, space="PSUM"))
        ps_s = ctx.enter_context(tc.tile_pool(name="ps_s", bufs=1, space="PSUM"))
        ps_agg = ctx.enter_context(tc.tile_pool(name="ps_agg", bufs=2, space="PSUM"))
        ps_lg = ctx.enter_context(tc.tile_pool(name="ps_lg", bufs=2, space="PSUM"))

        for g in range(G):
            # ---- loads ----
            xt = xt_pool.tile([FIN, N], F32, tag="xt")
            nc.sync.dma_start(xt[:], xT[g])
            adjp_t, adjm_t = [], []
            for jt in range(NT):
                ap_ = adj_pool.tile([128, N], BF16, tag="adjp")
                nc.sync.dma_start(ap_[:], adjP[g, jt * 128:(jt + 1) * 128, :])
                adjp_t.append(ap_)
                am_ = adj_pool.tile([128, N], BF16, tag="adjm")
                nc.sync.dma_start(am_[:], adjM[g, jt * 128:(jt + 1) * 128, :])
                adjm_t.append(am_)

            x1t = None
            for layer in range(2):
                # ---- projections: [hp | d | s] per node tile ----
                hp_aug, r_sc, ed2_sc = [], [], []
                for jt in range(NT):
                    pp = ps_proj.tile([128, F + 2 * H], F32, tag="proj")
                    if layer == 0:
                        nc.tensor.matmul(
                            pp[:], xt[:, jt * 128:(jt + 1) * 128], w0d_sb[:],
                            start=True, stop=True)
                    else:
                        for ct in range(NT):
                            nc.tensor.matmul(
                                pp[:], x1t[ct][:, jt * 128:(jt + 1) * 128],
                                w1d_sb[:, ct, :],
                                start=(ct == 0), stop=(ct == NT - 1))
                    ha = hp_pool.tile([128, F + 1], BF16, tag="hp")
                    nc.scalar.copy(ha[:, 0:F], pp[:, 0:F])
                    # ones column; 8.0 on layer 1 folds the head-mean into 1/den
                    nc.vector.memset(ha[:, F:F + 1], 1.0 if layer == 0 else 8.0)
                    hp_aug.append(ha)
                    rr = dsc_pool.tile([128, H], F32, tag="rsc")
                    nc.scalar.activation(rr[:], pp[:, F:F + H], AF.Exp, scale=0.8)
                    ee = dsc_pool.tile([128, H], F32, tag="ed2")
                    nc.scalar.activation(ee[:], pp[:, F:F + H], AF.Exp, scale=0.2)
                    r_sc.append(rr)
                    ed2_sc.append(ee)

                # s rows (and raw d rows for the logit path)
                ps = ps_s.tile([H, N], F32, tag="s")
                if layer == 0:
                    nc.tensor.matmul(ps[:], w0d_sb[:, F + H:F + 2 * H], xt[:],
                                     start=True, stop=True)
                else:
                    for ct in range(NT):
                        nc.tensor.matmul(
                            ps[:], w1d_sb[:, ct, F + H:F + 2 * H], x1t[ct][:],
                            start=(ct == 0), stop=(ct == NT - 1))
                # d^T rows (raw logit halves for the ACT path)
                pd = ps_s.tile([H, N], F32, tag="dT")
                if layer == 0:
                    nc.tensor.matmul(pd[:], w0d_sb[:, F:F + H], xt[:],
                                     start=True, stop=True)
                else:
                    for ct in range(NT):
                        nc.tensor.matmul(
                            pd[:], w1d_sb[:, ct, F:F + H], x1t[ct][:],
                            start=(ct == 0), stop=(ct == NT - 1))

                # rows: exp(s), exp(.2 s), raw s, raw d -> DRAM bounce
                esr = esr_pool.tile([H, 4 * N], BF16, tag="esr")
                nc.scalar.activation(esr[:, 0:N], ps[:], AF.Exp, scale=1.0)
                nc.scalar.activation(esr[:, N:2 * N], ps[:], AF.Exp, scale=0.2)
                nc.scalar.copy(esr[:, 2 * N:3 * N], ps[:])
                nc.scalar.copy(esr[:, 3 * N:4 * N], pd[:])
                esd = esd_pool.tile([H, 4 * N], BF16, tag="esd")
                nc.sync.dma_start(esd[:], esr[:])
                # flat single-partition views of raw s / d rows (matmul operands
                # must sit at base partition 0)
                stf = esr_pool.tile([1, H * N], BF16, tag="stf",
                                    name=f"stf_{g}_{layer}")
                nc.sync.dma_start(stf[:], esd[:, 2 * N:3 * N])
                dtf = esr_pool.tile([1, H * N], BF16, tag="dtf",
                                    name=f"dtf_{g}_{layer}")
                nc.sync.dma_start(dtf[:], esd[:, 3 * N:4 * N])

                es_bc, es2_bc = [], []
                for h in range(H):
                    eb = esbc_pool.tile([128, N], BF16, tag="esbc")
                    nc.sync.dma_start(
                        eb[:], esd[h:h + 1, 0:N].partition_broadcast(128))
                    es_bc.append(eb)
                    eb2 = esbc_pool.tile([128, N], BF16, tag="es2bc")
                    nc.sync.dma_start(
                        eb2[:], esd[h:h + 1, N:2 * N].partition_broadcast(128))
                    es2_bc.append(eb2)

                # destination of normalized per-head outputs
                stacked = []
                for it in range(NT):
                    st = x1_pool.tile([128, C1], BF16, tag="x1pre",
                                      name=f"x1pre_{g}_{layer}_{it}")
                    stacked.append(st)

                # ---- per head: E build + aggregation + normalize ----
                for h in range(H):
                    et_h, rhs_h = [], []
                    for jt in range(NT):
                        et = et_pool.tile([128, N], BF16, tag="et")
                        if _a_path(h, jt):
                            # logits + additive mask in PSUM, then 2 exps + max
                            px = ps_lg.tile([128, N], F32, tag="lg")
                            # x[j,i] = 1*s_h[i] + d_h[j]*1 + adjM[j,i]
                            nc.tensor.matmul(
                                px[:], ones_row[:, jt * 128:(jt + 1) * 128],
                                stf[:, h * N:(h + 1) * N],
                                start=True, stop=False)
                            nc.tensor.matmul(
                                px[:],
                                dtf[:, h * N + jt * 128:h * N + (jt + 1) * 128],
                                ones_row[:], start=False, stop=False)
                            nc.tensor.matmul(
                                px[:], id_sb[:], adjm_t[jt][:],
                                start=False, stop=True)
                            t1 = u_pool.tile([128, N], BF16, tag="u")
                            nc.scalar.activation(t1[:], px[:], AF.Exp)
                            t2 = u_pool.tile([128, N], BF16, tag="u")
                            nc.scalar.activation(t2[:], px[:], AF.Exp, scale=0.2)
                            nc.vector.tensor_tensor(
                                et[:], t1[:], t2[:], AluOpType.max)
                            rhs_h.append(hp_aug[jt])
                        else:
                            # rank-1 exp factors + per-partition scalars
                            hps = hps_pool.tile([128, F + 1], BF16, tag="hps")
                            nc.vector.tensor_scalar_mul(
                                hps[:], hp_aug[jt][:], ed2_sc[jt][:, h:h + 1])
                            rhs_h.append(hps)
                            w = u_pool.tile([128, N], BF16, tag="u")
                            nc.vector.tensor_scalar_mul(
                                w[:], es_bc[h][:], r_sc[jt][:, h:h + 1])
                            u = u_pool.tile([128, N], BF16, tag="u")
                            nc.vector.tensor_tensor(
                                u[:], w[:], es2_bc[h][:], AluOpType.max)
                            eng = nc.gpsimd if _gps_mask(h, jt) else nc.vector
                            eng.tensor_tensor(
                                et[:], u[:], adjp_t[jt][:], AluOpType.mult)
                        et_h.append(et)

                    po = ps_agg.tile([128, NT * (F + 1)], F32, tag="agg")
                    for it in range(NT):
                        for jt in range(NT):
                            nc.tensor.matmul(
                                po[:, it * (F + 1):(it + 1) * (F + 1)],
                                et_h[jt][:, it * 128:(it + 1) * 128],
                                rhs_h[jt][:],
                                start=(jt == 0), stop=(jt == NT - 1))

                    den = den_pool.tile([128, NT], F32, tag="den")
                    nc.scalar.copy(
                        den[:], po[:, F:F + 1 + (NT - 1) * (F + 1):F + 1])
                    rd = den_pool.tile([128, NT], F32, tag="rd")
                    nc.vector.reciprocal(rd[:], den[:])
                    for it in range(NT):
                        dst = stacked[it][:, h * F:(h + 1) * F]
                        src = po[:, it * (F + 1):it * (F + 1) + F]
                        if (h + it) % 2 == 0:
                            nc.scalar.activation(dst, src, AF.Copy,
                                                 scale=rd[:, it:it + 1])
                        else:
                            nc.vector.tensor_scalar_mul(dst, src,
                                                        rd[:, it:it + 1])

                # ---- post ----
                if layer == 0:
                    x1t = [x1t_pool.tile([128, N], BF16, tag="x1t",
                                         name=f"x1t_{g}_{ct}")
                           for ct in range(NT)]
                    for it in range(NT):
                        t = post_pool.tile([128, C1], BF16, tag="expt")
                        nc.scalar.activation(t[:], stacked[it][:], AF.Exp)
                        u2 = post_pool.tile([128, C1], BF16, tag="u2")
                        nc.vector.tensor_scalar_add(u2[:], t[:], -1.0)
                        x1e = post_pool.tile([128, C1], BF16, tag="x1e")
                        # elu(x) = min(relu(x), exp(x) - 1)
                        nc.vector.scalar_tensor_tensor(
                            x1e[:], stacked[it][:], 0.0, u2[:],
                            AluOpType.max, AluOpType.min)
                        for ct in range(NT):
                            nc.sync.dma_start_transpose(
                                x1t[ct][:, it * 128:(it + 1) * 128],
                                x1e[:, ct * 128:(ct + 1) * 128])
                else:
                    for it in range(NT):
                        t1 = out_pool.tile([128, C1 // 2], F32, tag="t1")
                        nc.vector.tensor_add(t1[:], stacked[it][:, 0:256],
                                             stacked[it][:, 256:512])
                        t2 = out_pool.tile([128, C1 // 4], F32, tag="t2")
                        nc.vector.tensor_add(t2[:], t1[:, 0:128], t1[:, 128:256])
                        oo = out_pool.tile([128, F], F32, tag="oo")
                        nc.vector.tensor_add(oo[:], t2[:, 0:64], t2[:, 64:128])
                        nc.sync.dma_start(out[g, it * 128:(it + 1) * 128, :], oo[:])


def _get_nc():
    if "nc" not in _cached:
        _cached["nc"] = _build()
    return _cached["nc"]


def _prep_inputs(x, adj, W0, a_src0, a_dst0, W1, a_src1, a_dst1):
    x = np.asarray(x, np.float32)
    adj = np.array(adj, np.float32, copy=True)
    idx = np.arange(N)
    adj[:, idx, idx] = 1.0  # self loops (reference mutates adj the same way)
    xT = np.ascontiguousarray(x.transpose(0, 2, 1))          # [B, 64, 512]
    adjPf = np.where(adj > 0, np.float32(1), np.float32(0)).astype(BF)
    adjMf = np.where(adj > 0, np.float32(0), np.float32(-BIG)).astype(BF)
    identf = np.eye(128, dtype=np.float32).astype(BF)
    W0 = np.asarray(W0, np.float32)
    W1 = np.asarray(W1, np.float32)
    w0d = np.concatenate(
        [W0, W0 @ np.asarray(a_dst0, np.float32),
         W0 @ np.asarray(a_src0, np.float32)], axis=1)
    w1d = np.concatenate(
        [W1, W1 @ np.asarray(a_dst1, np.float32),
         W1 @ np.asarray(a_src1, np.float32)], axis=1).astype(BF)
    in_maps = []
    for c in range(NCORES):
        sl = slice(c * G, (c + 1) * G)
        in_maps.append(dict(
            xT=np.ascontiguousarray(xT[sl]),
            adjP=np.ascontiguousarray(adjPf[sl]),
            adjM=np.ascontiguousarray(adjMf[sl]),
            ident=identf, w0d=w0d, w1d=w1d,
        ))
    return in_maps


def run(inputs, **kw):
    """Build+run; returns (output [B,N,F] float32, BassKernelResults)."""
    nc = _get_nc()
    in_maps = _prep_inputs(
        inputs["x"], inputs["adj"], inputs["W0"], inputs["a_src0"],
        inputs["a_dst0"], inputs["W1"], inputs["a_src1"], inputs["a_dst1"])
    res = run_bass_kernel_spmd(nc, in_maps, list(range(NCORES)), **kw)
    outs = [res.results[c]["out"].reshape(G, N, F) for c in range(NCORES)]
    return np.concatenate(outs, axis=0).astype(np.float32), res


def kernel(**inputs):
    out, _ = run(inputs)
    return out
